# revision 5
# baseline (speedup 1.0000x reference)
"""CBOW negative-sampling loss kernel for Trainium2 (8 NeuronCores, SPMD).

Per batch element b: gather 21 rows of 50 floats (10 ctx rows from in_embed,
1 pos + 10 neg from out_embed), context sum, 11 dot products, log-sigmoids,
global mean.

v2: bulk gathers via the extended-ISA `dma_gather` (InstDMAGatherAnt)
instead of per-row indirect_dma_start.  One gather instruction fetches
thousands of random table rows (one SDMA descriptor per row), so the
~1.8us/op SWDGE fixed cost of the v1 kernel (2688 ops/core -> 4.8ms)
amortizes away; the cost model pegs the descriptor drain at ~0.5ms/core.

dma_gather constraints (HW-verified):
- int16 indices, gathered element must be a multiple of 256B.  VOCAB=50000
  rows don't fit int16, so the table is stored fp16 with rows padded to
  128B and viewed as [25000, 256B] blocks: block = v>>1 (max 24999), the
  wanted row is the (v&1) half.  The half-select runs on DVE against a
  host-uploaded 0/1 mask (only the 50 useful lanes are touched).
- single_packet=True caps a call at 64 descs/engine (1024 idxs); pass
  single_packet=False for larger calls.
- Index list for a call is wrapped into 16 partitions ([16, n/16],
  idx_list[i] at [i%16, i//16]) and replicated 8x to 128 partitions.
- Gathered element i lands at partition i%128, free column i//128.

Per group of T=8 tiles (1024 batch elems) two gathers fetch 10*T*128 ctx
blocks and 11*T*128 pos/neg blocks as [128, slots, 128f16] tiles; DVE does
sel = lo + mask*(hi-lo), a 10->1 context tree sum, products against the
broadcast ctx vector, and a 50->1 reduce into fp32 scores.  Tail as v1:
-log sig(+-s) == softplus(-+s) == Ln(1+Exp(-+0.1*s)) (the 0.1 folds the
/10 context mean), Ln's accum_out yields per-partition loss sums.
Host: loss = +(sum of partials) / B.
"""

import sys

import numpy as np

if "/opt/trn_rl_repo" not in sys.path:
    sys.path.insert(0, "/opt/trn_rl_repo")

from concourse import bass, mybir  # noqa: E402
from concourse import bass_utils  # noqa: E402
from concourse import tile  # noqa: E402
from concourse.bacc import Bacc  # noqa: E402

VOCAB = 50000
DIM = 50
B = 131072
CTX = 10
NEG = 10

NCORES = 8
P = 128
BC = B // NCORES  # 16384
NTILES = BC // P  # 128
NBLK = VOCAB // 2  # 25000 gather blocks per table
ELEM = 128  # f16 per gather block (256B)
T = 4  # tiles per gather group
QUEUES = 4  # SWDGE queues (parallel Q7 desc-gen core pairs)
CW = CTX * T * P // 16  # idx cols per ctx call (640)
OW = (NEG + 1) * T * P // 16  # idx cols per out call (704)
GW = CW + OW  # idx cols per group (1344)

f16 = mybir.dt.float16
f32 = mybir.dt.float32
i16 = mybir.dt.int16


def build_nc(ntiles: int = NTILES, repeats: int = 1, dump_scores: bool = False):
    nc = Bacc(None, target_bir_lowering=False, num_swdge_queues=QUEUES)
    one_t = nc.alloc_sbuf_tensor("const-one", [P, 1], f32)
    nc.gpsimd.memset(one_t.ap(), 1.0)
    nc.const_aps.aps[(f32, 1.0)] = one_t.ap()
    nc.all_engine_barrier()

    tin = nc.dram_tensor("tin", [NBLK, ELEM], f16, kind="ExternalInput")
    tout = nc.dram_tensor("tout", [NBLK, ELEM], f16, kind="ExternalInput")
    ngroups = ntiles // T
    assert ngroups * T == ntiles
    idx = nc.dram_tensor("idx", [P, ngroups * GW], i16, kind="ExternalInput")
    mkc = nc.dram_tensor("mkc", [P, ntiles * CTX], f16, kind="ExternalInput")
    mko = nc.dram_tensor(
        "mko", [P, ntiles * (NEG + 1)], f16, kind="ExternalInput"
    )
    partial = nc.dram_tensor("partial", [P, 1], f32, kind="ExternalOutput")
    scores_out = (
        nc.dram_tensor("scores_out", [P, ntiles * 11], f32, kind="ExternalOutput")
        if dump_scores
        else None
    )

    with tile.TileContext(nc) as tc:
        with (
            tc.tile_pool(name="idxp", bufs=1) as ipool,
            tc.tile_pool(name="gidx", bufs=4) as gipool,
            tc.tile_pool(name="gather", bufs=4) as gpool,
            tc.tile_pool(name="work", bufs=1) as wpool,
            tc.tile_pool(name="stage", bufs=1) as spool,
        ):
          for rep in range(repeats):
            mct = ipool.tile([P, ntiles * CTX], f16, tag="mct")
            nc.sync.dma_start(out=mct[:], in_=mkc[:])
            mcv = mct[:].rearrange(
                "p (g t s) -> p g t s", g=ngroups, t=T, s=CTX
            )
            mot = ipool.tile([P, ntiles * (NEG + 1)], f16, tag="mot")
            nc.sync.dma_start(out=mot[:], in_=mko[:])
            mov = mot[:].rearrange(
                "p (g t s) -> p g t s", g=ngroups, t=T, s=NEG + 1
            )

            scores = spool.tile([P, ntiles * 11], f32, tag="scores")
            scv = scores[:].rearrange(
                "p (g t j) -> p g t j", g=ngroups, t=T, j=11
            )

            for g in range(ngroups):
                gi = gipool.tile([P, GW], i16, tag="gi")
                nc.sync.dma_start(out=gi[:], in_=idx[:, g * GW : (g + 1) * GW])
                ct = gpool.tile([P, CTX * T * ELEM], f16, tag="ct")
                nc.gpsimd.dma_gather(
                    ct[:].rearrange("p (c d) -> p c d", c=CTX * T, d=ELEM),
                    tin[:],
                    gi[:, 0:CW],
                    CTX * T * P,
                    CTX * T * P,
                    ELEM,
                    single_packet=False,
                    queue_num=(2 * g) % QUEUES,
                )
                ot = gpool.tile([P, (NEG + 1) * T * ELEM], f16, tag="ot")
                nc.gpsimd.dma_gather(
                    ot[:].rearrange("p (c d) -> p c d", c=(NEG + 1) * T, d=ELEM),
                    tout[:],
                    gi[:, CW:GW],
                    (NEG + 1) * T * P,
                    (NEG + 1) * T * P,
                    ELEM,
                    single_packet=False,
                    queue_num=(2 * g + 1) % QUEUES,
                )
                cv = ct[:].rearrange(
                    "p (t s d) -> p t s d", t=T, s=CTX, d=ELEM
                )
                ov = ot[:].rearrange(
                    "p (t s d) -> p t s d", t=T, s=NEG + 1, d=ELEM
                )

                # ctx rows: sel = lo + m*(hi-lo), then 10 -> 1 tree sum
                dc = wpool.tile([P, T * CTX * DIM], f16, tag="dc")
                dcv = dc[:].rearrange(
                    "p (t s d) -> p t s d", t=T, s=CTX, d=DIM
                )
                nc.vector.tensor_sub(
                    out=dcv, in0=cv[:, :, :, 64 : 64 + DIM], in1=cv[:, :, :, 0:DIM]
                )
                mcb = mcv[:, g].unsqueeze(3).broadcast_to((P, T, CTX, DIM))
                nc.vector.tensor_mul(out=dcv, in0=dcv, in1=mcb)
                sc = wpool.tile([P, T * CTX * DIM], f16, tag="sc")
                scv4 = sc[:].rearrange(
                    "p (t s d) -> p t s d", t=T, s=CTX, d=DIM
                )
                nc.vector.tensor_add(out=scv4, in0=dcv, in1=cv[:, :, :, 0:DIM])
                s5 = wpool.tile([P, T * 5 * DIM], f16, tag="s5")
                s5v = s5[:].rearrange("p (t s d) -> p t s d", t=T, s=5, d=DIM)
                nc.vector.tensor_add(
                    out=s5v, in0=scv4[:, :, 0:5], in1=scv4[:, :, 5:10]
                )
                s2 = wpool.tile([P, T * 2 * DIM], f16, tag="s2")
                s2v = s2[:].rearrange("p (t s d) -> p t s d", t=T, s=2, d=DIM)
                nc.vector.tensor_add(
                    out=s2v, in0=s5v[:, :, 0:2], in1=s5v[:, :, 2:4]
                )
                s1 = wpool.tile([P, T * 1 * DIM], f16, tag="s1")
                s1v = s1[:].rearrange("p (t s d) -> p t s d", t=T, s=1, d=DIM)
                nc.vector.tensor_add(
                    out=s1v, in0=s2v[:, :, 0:1], in1=s2v[:, :, 1:2]
                )
                ctx = wpool.tile([P, T * 1 * DIM], f16, tag="ctx")
                ctxv = ctx[:].rearrange("p (t s d) -> p t s d", t=T, s=1, d=DIM)
                nc.vector.tensor_add(out=ctxv, in0=s1v, in1=s5v[:, :, 4:5])

                # pos/neg rows: sel, then dot with broadcast ctx
                do = wpool.tile([P, T * (NEG + 1) * DIM], f16, tag="do")
                dov = do[:].rearrange(
                    "p (t s d) -> p t s d", t=T, s=NEG + 1, d=DIM
                )
                nc.vector.tensor_sub(
                    out=dov, in0=ov[:, :, :, 64 : 64 + DIM], in1=ov[:, :, :, 0:DIM]
                )
                mob = mov[:, g].unsqueeze(3).broadcast_to((P, T, NEG + 1, DIM))
                nc.vector.tensor_mul(out=dov, in0=dov, in1=mob)
                so = wpool.tile([P, T * (NEG + 1) * DIM], f16, tag="so")
                sov = so[:].rearrange(
                    "p (t s d) -> p t s d", t=T, s=NEG + 1, d=DIM
                )
                nc.vector.tensor_add(out=sov, in0=dov, in1=ov[:, :, :, 0:DIM])
                ctxb = ctxv.broadcast_to((P, T, NEG + 1, DIM))
                nc.vector.tensor_mul(out=sov, in0=sov, in1=ctxb)
                nc.vector.tensor_reduce(
                    out=scv[:, g, :, :],
                    in_=sov,
                    axis=mybir.AxisListType.X,
                    op=mybir.AluOpType.add,
                    negate=False,
                )

            acc = spool.tile([P, 1], f32, tag="acc")
            if dump_scores:
                nc.sync.dma_start(out=scores_out[:], in_=scores[:])
            sall = scores[:].rearrange("p (t j) -> p t j", t=ntiles, j=11)
            # -log sig(pos_s) = softplus(-pos_s), -log sig(-neg_s) =
            # softplus(neg_s); softplus(x) = Ln(1 + Exp(x)) keeps both
            # activations in the natural_log_exp table set (one load).
            nc.scalar.activation(
                out=sall[:, :, 0:1],
                in_=sall[:, :, 0:1],
                func=mybir.ActivationFunctionType.Exp,
                scale=-0.1,
            )
            nc.scalar.activation(
                out=sall[:, :, 1:11],
                in_=sall[:, :, 1:11],
                func=mybir.ActivationFunctionType.Exp,
                scale=0.1,
            )
            nc.scalar.activation(
                out=scores[:],
                in_=scores[:],
                func=mybir.ActivationFunctionType.Ln,
                bias=1.0,
                accum_out=acc[:],
            )
            nc.sync.dma_start(out=partial[:], in_=acc[:])

    nc.compile()
    return nc


def _wrap16(flat: np.ndarray) -> np.ndarray:
    """idx_list[i] -> [i%16, i//16], replicated to 128 partitions."""
    w = flat.reshape(-1, 16).T
    return np.tile(w, (8, 1))


def _prep_inputs(context_idxs, pos_target, neg_samples, in_embed_W, out_embed_W):
    ci = np.asarray(context_idxs, dtype=np.int64)  # [B, 10]
    po = np.concatenate(
        [
            np.asarray(pos_target, dtype=np.int64)[:, None],
            np.asarray(neg_samples, dtype=np.int64),
        ],
        axis=1,
    )  # [B, 11]

    def pack(w):
        t = np.zeros((VOCAB, 64), dtype=np.float16)
        t[:, :DIM] = np.asarray(w).astype(np.float16)
        return t.reshape(NBLK, ELEM)

    tin = pack(in_embed_W)
    tout = pack(out_embed_W)

    ngroups = NTILES // T
    in_maps = []
    for c in range(NCORES):
        cic = ci[c * BC : (c + 1) * BC]
        poc = po[c * BC : (c + 1) * BC]
        bc = (cic >> 1).astype(np.int16).reshape(ngroups, T, P, CTX)
        bo = (poc >> 1).astype(np.int16).reshape(ngroups, T, P, NEG + 1)
        cols = []
        for g in range(ngroups):
            cols.append(_wrap16(bc[g].transpose(0, 2, 1).reshape(-1)))
            cols.append(_wrap16(bo[g].transpose(0, 2, 1).reshape(-1)))
        idx_c = np.concatenate(cols, axis=1)  # [128, ngroups*GW]
        mkc = (
            (cic & 1)
            .astype(np.float16)
            .reshape(NTILES, P, CTX)
            .transpose(1, 0, 2)
            .reshape(P, NTILES * CTX)
            .copy()
        )
        mko = (
            (poc & 1)
            .astype(np.float16)
            .reshape(NTILES, P, NEG + 1)
            .transpose(1, 0, 2)
            .reshape(P, NTILES * (NEG + 1))
            .copy()
        )
        in_maps.append(
            {"tin": tin, "tout": tout, "idx": idx_c, "mkc": mkc, "mko": mko}
        )
    return in_maps


def kernel(context_idxs, pos_target, neg_samples, in_embed_W, out_embed_W):
    in_maps = _prep_inputs(
        context_idxs, pos_target, neg_samples, in_embed_W, out_embed_W
    )
    nc = build_nc()
    res = bass_utils.run_bass_kernel_spmd(nc, in_maps, core_ids=list(range(NCORES)))
    # partials are sums of softplus terms = -(log-sigmoid sums), so the
    # loss is +total/B
    total = sum(float(r["partial"].sum()) for r in res.results)
    return np.float32(total / B)
